# revision 1
# baseline (speedup 1.0000x reference)
"""Trainium2 Bass kernel for nn_Attention1 (dense transformer attention with
query-summed output).

Reference computation (per batch b):
    query  = x * drop_mask                       [S, D]
    scores = query @ x.T / sqrt(D)               [S, S]
    att    = softmax(scores, axis=-1)
    out[b] = (att @ x).sum(axis=queries)         [D]

Key identity: out[b] = w @ x where w[k] = sum_q att[q, k] (attention column
sums), so the full PV matmul is never needed — only the column sums of the
softmax matrix.

Sharding: pure data parallel, batch B=8 across the 8 NeuronCores.

v2 design (per core, S=4096, D=256):
  Phase A:
    - x: bulk f32->f16 DRAM->DRAM SWDGE cast-DMA, then TWO full-S XBAR
      transpose loads (one per d-half) on the sync ring only. Each DMA
      transpose costs ~8-12us of ring time regardless of size and
      serializes against all outstanding DMAs, so: fewest possible, one
      ring, nothing else queued around them.
    - mask: loaded directly as f32 rows (no cast); q16 = (x/16)*m on DVE.
    - q-side transposed on the PE (idle during phase A, no DMA involved):
      64 transpose-mode matmuls vs an identity + DVE PSUM->SBUF copies.
    - fp8e4 operands qT8 = (x*m/4)^T, xT8 = (x/4)^T (so s = q.x/16 with both
      operands in fp8 normal range).
    - Diagonal s_qq via DVE in row layout -> bias = 8*ln2 - diag.
  Phase B (32 stripes of 128 queries, 4 k-slices of 1024, 3 PSUM bufs):
    - fp8 DoubleRow matmuls (K=256 in one MM): 8 MMs of N=512 per stripe.
    - Slices 0-2 on ScalarE: EXP with per-row bias + fused row-sum accum.
    - Slice 3 on DVE: Schraudolph bit-exp, uint16 <- sat(round(s*C1+bias2));
      the uint16 pattern IS fp16 e (negatives saturate to 0 = +0.0). Written
      to a u16 scratch then reinterpret-copied (same-engine) into et2, plus a
      tensor_reduce row-sum partial. Separate tiles from the ScalarE slices.
    - Column sums w += r_q e[q, :] as M=1 matvecs, 4 col-strips packed per
      PSUM bank via tile_position, accumulated across all 32 stripes in two
      persistent PSUM banks.
  Tail: W -> SBUF, 32 K=1 transpose MMs -> w in partition layout,
    out = w16 @ x16 (32 accumulating fp16 MMs), copy, DMA out.
"""

import os
import sys

import numpy as np

_TRN_REPO = "/opt/trn_rl_repo"
if os.path.isdir(_TRN_REPO) and _TRN_REPO not in sys.path:
    sys.path.insert(0, _TRN_REPO)

import concourse.bass as bass
import concourse.mybir as mybir
import concourse.tile as tile
from concourse import bacc, masks
from concourse.bass_utils import run_bass_kernel_spmd

F32 = mybir.dt.float32
F16 = mybir.dt.float16
U16 = mybir.dt.uint16
F8 = mybir.dt.float8e4
DR = mybir.MatmulPerfMode.DoubleRow
ALU = mybir.AluOpType

B = 8
S = 4096
D = 256
P = 128

NST = S // P          # 32 query stripes of 128 rows
NSB = S // 512        # 8 blocks of 512 rows (4 stripes)
E_SHIFT = float(8 * np.log(2.0))  # exp() output centering: diagonal -> 2^8
C1 = float(1024.0 / np.log(2.0))  # bit-exp scale (fp16 mantissa bits)
C2 = float(15 * 1024 - 0.0430 * 1024)  # fp16 exp bias - Schraudolph correction

# k-slices for the scores PSUM tiles: 1024-wide tiles (2 banks each, 3 bufs)
K_SLICES = [(0, 1024), (1024, 1024), (2048, 1024), (3072, 1024)]



def build_kernel(finalize: bool = True) -> bass.Bass:
    nc = bacc.Bacc(None)

    x_in = nc.declare_dram_parameter("x", [S, D], F32, isOutput=False)
    m_in = nc.declare_dram_parameter("mask", [S, D], F32, isOutput=False)
    out_ext = nc.declare_dram_parameter("out", [1, D], F32, isOutput=True)

    x_in_t = x_in.rearrange("(a p) d -> p a d", p=P)      # [128, 32, 256]
    m_in_t = m_in.rearrange("(a p) d -> p a d", p=P)

    with tile.TileContext(nc) as tc:
        with (
            tc.tile_pool(name="dram", bufs=1, space="DRAM") as dramp,
            tc.tile_pool(name="res", bufs=1) as res,
            tc.tile_pool(name="etile", bufs=8) as ep,
            tc.tile_pool(name="small", bufs=8) as smallp,
            tc.tile_pool(name="eu", bufs=3) as ep2,
            tc.tile_pool(name="e2", bufs=8) as ep3,
            tc.tile_pool(name="ps_scores", bufs=3, space="PSUM") as pss,
            tc.tile_pool(name="ps_misc", bufs=2, space="PSUM") as psm,
        ):
            # DRAM bounce buffers (fp16) for the XBAR transposes.
            # Split per half-tensor: DRAM tile deps are tile-granular, so a
            # single [S, D] bounce would serialize every consumer behind the
            # LAST producer chunk.
            x16d = dramp.tile([S, D], F16)

            # SBUF residents
            xT16 = res.tile([P, 2, S], F16)   # x^T fp16 [d%128, d//128, s]
            qT16 = res.tile([P, 2, S], F16)   # (x*m/16)^T fp16
            qT8 = res.tile([P, 2, S], F8)     # (x*m/4)^T fp8
            xT8 = res.tile([P, 2, S], F8)     # (x/4)^T fp8
            x16 = res.tile([P, NST, D], F16)  # x fp16 row layout
            bias_all = res.tile([P, NST], F32)    # E_SHIFT - diag
            bias2_all = res.tile([P, NST], F32)   # bias_all*C1 + C2 (bit-exp)
            ones = res.tile([P, 1], F32)
            ones16 = res.tile([P, 1], F16)
            wtot16 = res.tile([P, NST], F16)
            out_sb = res.tile([1, D], F32)

            ident = res.tile([P, P], F16)
            nc.vector.memset(ones[:], 1.0)
            nc.vector.memset(ones16[:], 1.0)
            masks.make_identity(nc, ident[:])


            # ---- Phase A ----
            # Bulk f32->f16 casts run as DRAM->DRAM SWDGE cast-DMAs (2 chunks
            # each: SWDGE dispatch is ~3.7us/DMA, so few + fat). The XBAR
            # transposes read the casted bounces; row-layout fp16 loads feed
            # q16/diag on DVE.
            NCH = 8  # 512-row chunks
            rows_per = S // NCH
            a_per = rows_per // P
            x16dv = x16d.rearrange("(a p) d -> p a d", p=P)
            # x casts first: they gate the xT transposes and hence phase B.
            # m casts are emitted AFTER the xT transposes: each DMA transpose
            # serializes against all outstanding DMAs (deadlock guard), so
            # nothing else may be in flight while the xT batch runs.
            for h in range(2):
                hr = slice(h * (S // 2), (h + 1) * (S // 2))
                nc.gpsimd.dma_start(x16d[hr, :], x_in[hr, :])
            def emit_xT(d):
                # one full-S transpose per d-half: each DMA transpose costs
                # ~8-12us of ring time regardless of size, so fewest wins
                nc.sync.dma_start(
                    xT16[:, d, :], x16d[:, d * P : (d + 1) * P], transpose=True
                )

            def emit_xT8(d):
                nc.vector.tensor_scalar(
                    xT8[:, d, :], xT16[:, d, :], 0.25, None, ALU.mult
                )

            def emit_qT8(h):
                rows2 = slice(h * (S // 2), (h + 1) * (S // 2))
                nc.vector.tensor_scalar(
                    qT8[:, :, rows2], qT16[:, :, rows2], 4.0, None, ALU.mult
                )

            with tc.tile_pool(name="stage", bufs=3) as stage:
                for ch in range(NCH):
                    a0 = ch * a_per
                    rows = slice(ch * rows_per, (ch + 1) * rows_per)
                    asl = slice(a0, a0 + a_per)
                    m32c = stage.tile([P, a_per, D], F32, tag="m32")
                    x32c = stage.tile([P, a_per, D], F32, tag="x32")
                    q16c = stage.tile([P, a_per, D], F16, tag="q16")
                    # load x rows as f32 and cast on DVE: decouples the q-side
                    # pipeline from the x cast-DMA (which only feeds the XBAR)
                    nc.scalar.dma_start(x32c[:], x_in_t[:, asl, :])
                    nc.scalar.dma_start(m32c[:], m_in_t[:, asl, :])
                    nc.vector.tensor_copy(x16[:, asl, :], x32c[:])
                    # the xT XBAR batch goes out after the first chunks' loads
                    # so those loads aren't stuck behind the transpose guard
                    if ch == 2:
                        emit_xT(0)
                        emit_xT(1)
                    # q16 = (x/16)*m
                    nc.vector.scalar_tensor_tensor(
                        out=q16c[:], in0=x16[:, asl, :], scalar=1.0 / 16.0,
                        in1=m32c[:], op0=ALU.mult, op1=ALU.mult,
                    )
                    # q-side transpose on the PE (idle in phase A; no DMA, so
                    # it overlaps the xT batch instead of queueing behind it).
                    # 4 transposes batch into one PSUM tile -> one wide copy.
                    for d in range(2):
                        pt4 = psm.tile([P, 4 * P], F16, tag="a")
                        for a in range(a_per):
                            nc.tensor.transpose(
                                pt4[:, a * P : (a + 1) * P],
                                q16c[:, a, d * P : (d + 1) * P],
                                ident[:],
                            )
                        nc.vector.tensor_copy(qT16[:, d, rows], pt4[:])
                    nc.vector.tensor_scalar(
                        qT8[:, :, rows], qT16[:, :, rows], 4.0, None, ALU.mult
                    )
                    if ch == 4:
                        emit_xT8(0)
                    if ch == 6:
                        emit_xT8(1)
                    # diagonal on DVE in row layout: s_qq = sum_d q16*x16
                    tdiag = stage.tile([P, a_per, D], F16, tag="td")
                    nc.vector.tensor_tensor(tdiag[:], q16c[:], x16[:, asl, :], ALU.mult)
                    bsl = slice(4 * ch, 4 * ch + 4)
                    dcol = stage.tile([P, a_per], F32, tag="dc")
                    nc.vector.tensor_reduce(dcol[:], tdiag[:], mybir.AxisListType.X, ALU.add)
                    nc.vector.tensor_scalar(
                        bias_all[:, bsl], dcol[:], -1.0, E_SHIFT, ALU.mult, ALU.add
                    )
                    nc.vector.tensor_scalar(
                        bias2_all[:, bsl], bias_all[:, bsl], C1, C2, ALU.mult, ALU.add
                    )

            # colsum accumulation: persistent PSUM groups across all blocks
            wsum = res.tile([P, 2, 512], F32)
            W0 = psm.tile([P, 512], F32, tag="a")
            W1 = psm.tile([P, 512], F32, tag="a")
            Wt = (W0, W1)

            def emit_colsum(blk, e_tiles, rb):
                first = blk == 0
                last = blk == NSB - 1
                for j in range(4):
                    for g in range(2):
                        for c in range(4):
                            ks = g * 4 + c
                            et, et2 = e_tiles[j]
                            if ks < 6:
                                rhs = et[:, ks * 512 : (ks + 1) * 512]
                            else:
                                rhs = et2[:, (ks - 6) * 512 : (ks - 5) * 512]
                            nc.tensor.matmul(
                                Wt[g][32 * c : 32 * c + 1, :],
                                lhsT=rb[:, j : j + 1],
                                rhs=rhs,
                                start=(first and j == 0),
                                stop=(last and j == 3),
                                tile_position=(0, 32 * c),
                                skip_group_check=True,
                            )
                if last:
                    for g in range(2):
                        nc.vector.tensor_copy(wsum[:, g, :], Wt[g][:])

            # ---- Phase B ----
            def finalize_r(zpb, zvb, rb):
                # r = fp16(1 / (scalar partials + dve partial)) for 4 stripes
                zs = smallp.tile([P, 4], F32, tag="zs")
                nc.vector.tensor_reduce(zs[:], zpb[:], mybir.AxisListType.X, ALU.add)
                nc.vector.tensor_tensor(zs[:], zs[:], zvb[:], ALU.add)
                nc.vector.reciprocal(zs[:], zs[:])
                nc.vector.tensor_copy(rb[:], zs[:])

            prev = None
            for blk in range(NSB):
                e_tiles = []
                zpb = smallp.tile([P, 4, 3], F32, tag="z")
                zvb = smallp.tile([P, 4], F32, tag="zv")
                rb = smallp.tile([P, 4], F16, tag="r")
                for j in range(4):
                    qs = blk * 4 + j
                    et = ep.tile([P, 3072], F16, tag="e")
                    for ksl, (k0, kn) in enumerate(K_SLICES):
                        ps = pss.tile([P, 1024], F32, tag="s")
                        for n in range(kn // 512):
                            nc.tensor.matmul(
                                ps[:, n * 512 : (n + 1) * 512],
                                lhsT=qT8[:, :, qs * P : (qs + 1) * P],
                                rhs=xT8[:, :, k0 + n * 512 : k0 + (n + 1) * 512],
                                start=True,
                                stop=True,
                                perf_mode=DR,
                            )
                        if ksl == 3:
                            # DVE slice: bit-trick exp. DVE-written data lives
                            # in its own tiles (et2/zvb): sharing a tile with
                            # ScalarE writers races PE readers; and a bitcast
                            # OUT AP on the producer breaks dep tracking, so
                            # write u16 scratch then same-engine reinterpret
                            # copy.
                            eu = ep2.tile([P, 1024], U16, tag="eu")
                            nc.vector.tensor_scalar(
                                eu[:],
                                ps[:, :kn],
                                C1,
                                bias2_all[:, qs : qs + 1],
                                ALU.mult,
                                ALU.add,
                            )
                            et2 = ep3.tile([P, 1024], F16, tag="e2")
                            nc.vector.tensor_copy(et2[:], eu[:].bitcast(F16))
                            nc.vector.tensor_reduce(
                                zvb[:, j : j + 1],
                                et2[:],
                                mybir.AxisListType.X,
                                ALU.add,
                            )
                        else:
                            if ksl == 3:
                                et2 = ep3.tile([P, 1024], F16, tag="e2")
                                dst = et2[:, 0:kn]
                                acc = zvb[:, j : j + 1]
                            else:
                                dst = et[:, k0 : k0 + kn]
                                acc = zpb[:, j, :][:, ksl : ksl + 1]
                            nc.scalar.activation(
                                out=dst,
                                in_=ps[:, :kn],
                                func=mybir.ActivationFunctionType.Exp,
                                bias=bias_all[:, qs : qs + 1],
                                scale=1.0,
                                accum_out=acc,
                            )
                    e_tiles.append((et, et2))
                    # defer the previous block's r-finalize AND colsums past
                    # this block's first stripe, so the next bit-exp frees its
                    # scores buffer before the DVE runs the small-op chain
                    if j == 0 and prev is not None:
                        finalize_r(*prev[1])
                        emit_colsum(blk - 1, prev[0], prev[1][2])
                        prev = None
                prev = (e_tiles, (zpb, zvb, rb))
            finalize_r(*prev[1])
            emit_colsum(NSB - 1, prev[0], prev[1][2])

            # ---- Tail ----
            wtotP = psm.tile([P, NST], F32, tag="a")
            for i in range(NST):
                g, c, t0 = i // 16, (i % 16) // 4, (i % 4) * P
                nc.tensor.matmul(
                    wtotP[:, i : i + 1],
                    lhsT=wsum[:, g, t0 : t0 + P][32 * c : 32 * c + 1, :],
                    rhs=ones[32 * c : 32 * c + 1, :],
                    start=True,
                    stop=True,
                    tile_position=(32 * c, 0),
                )
            nc.vector.tensor_copy(wtot16[:], wtotP[:])
            po = psm.tile([1, D], F32, tag="a")
            for c in range(NST):
                nc.tensor.matmul(
                    po[:],
                    lhsT=wtot16[:, c : c + 1],
                    rhs=x16[:, c, :],
                    start=(c == 0),
                    stop=(c == NST - 1),
                )
            nc.vector.tensor_copy(out_sb[:], po[:])
            nc.sync.dma_start(out_ext[:, :], out_sb[:])

    if finalize:
        nc.finalize()
    return nc


def _run(x: np.ndarray, drop_mask: np.ndarray, trace: bool = False, nc=None):
    if nc is None:
        nc = build_kernel()
    in_maps = [{"x": x[b], "mask": drop_mask[b]} for b in range(B)]
    res = run_bass_kernel_spmd(nc, in_maps, list(range(B)), trace=trace)
    out = np.stack([res.results[b]["out"].reshape(D) for b in range(B)])
    return out.astype(np.float32), res


def kernel(**inputs: np.ndarray) -> np.ndarray:
    x = np.ascontiguousarray(inputs["x"], dtype=np.float32)
    drop_mask = np.ascontiguousarray(inputs["drop_mask"], dtype=np.float32)
    assert x.shape == (B, S, D) and drop_mask.shape == (B, S, D)
    out, _ = _run(x, drop_mask)
    return out


def profile(**inputs: np.ndarray):
    x = np.ascontiguousarray(inputs["x"], dtype=np.float32)
    drop_mask = np.ascontiguousarray(inputs["drop_mask"], dtype=np.float32)
    out, res = _run(x, drop_mask, trace=True)
    return res.exec_time_ns


if __name__ == "__main__":
    rng = np.random.default_rng(0)
    x = rng.standard_normal((B, S, D)).astype(np.float32)
    m = (rng.random((B, S, D)) < 0.5).astype(np.float32) * 2.0
    out = kernel(x=x, drop_mask=m)
    print(out.shape, out.dtype)

